# revision 1
# baseline (speedup 1.0000x reference)
"""Trainium2 Bass kernel for nn_DepthEstimationNet (vq_codebook).

reference:  d = x.reshape(B, S);  ratio[b,i,j] = d[b,i] * (1/d[b,j])
            out[b,i,j] = inv[searchsorted(q, ratio, side='right')]
shapes:     x [8,1,48,48] -> out [8, 2304, 2304] fp32 (~170 MB)

Strategy (data-parallel over batch, one batch per NeuronCore):
  - host computes recip = fl32(1/d) per batch (bit-identical to the
    reference's fp32 divide) and replicates it across 128 SBUF partitions.
  - per 128-row tile: v = d_col * recip (same fp32 rounding as the
    reference ratio), then a 40-step select-chain
        s = select(v >= q_k, inv[k+1], s)
    via a custom DVE op. Exact: compares are exact, values are copied.
  - row tiles are processed in groups of 3-4 with one wide DVE op per
    chain step ([128, W*2304]) to amortize per-instruction overhead.
  - q/inv are instruction immediates (same for all cores -> SPMD NEFF).
"""
import numpy as np

S = 2304          # 48*48
P = 128           # partitions
NT = S // P       # 18 row tiles per batch
NB = 40           # thresholds
B = 8             # batch == cores
GROUPS = (4, 4, 4, 3, 3)   # tile-group widths, sum = NT
WMAX = max(GROUPS)

_CACHE = {}


def _register_ops():
    import dataclasses
    import concourse.dve_ops as dve_ops_mod
    from concourse.dve_spec import Spec, Src0, Src1, C0, C1, C2, select
    from concourse.dve_ops import DveOp, OPS
    from concourse.dve_table_gen import dve_ver_for

    def reg(name, spec):
        for op in OPS:
            if op.name == name:
                return op
        op = DveOp(name, spec, subdim=False, uops_sha={})
        OPS.append(op)
        dve_ops_mod._SUB_OPCODE_FOR_NAME[name] = (
            dve_ops_mod._CUSTOM_DVE_ROW_BASE + len(OPS) - 1
        )
        assert dve_ops_mod._SUB_OPCODE_FOR_NAME[name] < 0x20
        dve_ops_mod.CUSTOM_DVE_SPECS[name] = spec
        ver = dve_ver_for("TRN2")
        try:
            op.compile(ver)
            return op
        except ValueError as e:
            import re
            m = re.search(r'uops_sha\["' + ver + r'"\]="([0-9a-f]+)"', str(e))
            assert m, f"no sha in: {e}"
            op2 = dataclasses.replace(op, uops_sha={ver: m.group(1)})
            OPS[OPS.index(op)] = op2
            return op2

    selchain = reg("ANT_SELCHAIN", Spec(body=select(Src0 >= C0, C1, Src1)))
    selinit = reg("ANT_SELINIT", Spec(body=select(Src0 >= C0, C1, C2)))
    return selchain, selinit


def _build_nc(q, inv, repeat=1, tiny_out=False):
    import concourse.bass as bass
    import concourse.mybir as mybir

    SELCHAIN, SELINIT = _register_ops()
    f32 = mybir.dt.float32

    nc = bass.Bass()
    r_in = nc.declare_dram_parameter("recipb", [P, S], f32, isOutput=False)
    d_in = nc.declare_dram_parameter("dcol", [P, NT], f32, isOutput=False)
    out_shape = [P, 8] if tiny_out else [S, S]
    y_out = nc.declare_dram_parameter("out", out_shape, f32, isOutput=True)

    NG = len(GROUPS)
    with (
        nc.sbuf_tensor("rb", [P, S], f32) as rb,
        nc.sbuf_tensor("dc", [P, NT], f32) as dc,
        nc.sbuf_tensor("v", [P, WMAX * S], f32) as v,
        nc.sbuf_tensor("x", [P, WMAX * S], f32) as x,
        nc.sbuf_tensor("y0", [P, WMAX * S], f32) as y0,
        nc.sbuf_tensor("y1", [P, WMAX * S], f32) as y1,
        nc.Block() as block,
        nc.semaphore("in_sem") as in_sem,
        nc.semaphore("grp_done") as grp_done,
        nc.semaphore("out_sem") as out_sem,
    ):
        ys = (y0, y1)

        @block.sync
        def _(sync):
            sync.dma_start(out=rb[:], in_=r_in[:]).then_inc(in_sem, 16)
            sync.dma_start(out=dc[:], in_=d_in[:]).then_inc(in_sem, 16)
            if tiny_out:
                sync.wait_ge(grp_done, NG * repeat)
                sync.dma_start(out=y_out[:], in_=y0[:, 0:8]).then_inc(out_sem, 16)
                sync.wait_ge(out_sem, 16)
            else:
                row0 = 0
                for g, W in enumerate(GROUPS):
                    sync.wait_ge(grp_done, g + 1)
                    dst = y_out[row0:row0 + W * P, :].rearrange(
                        "(w p) s -> p w s", p=P
                    )
                    src = ys[g % 2][:, 0:W * S].rearrange(
                        "p (w s) -> p w s", s=S
                    )
                    sync.dma_start(out=dst, in_=src).then_inc(out_sem, 16)
                    row0 += W * P
                sync.wait_ge(out_sem, 16 * NG)

        @block.vector
        def _(vector):
            vector.wait_ge(in_sem, 32)
            import contextlib
            rep_ctx = (
                vector.Fori(0, repeat) if repeat > 1 else contextlib.nullcontext()
            )
            with rep_ctx:
                t0 = 0
                for g, W in enumerate(GROUPS):
                    M = W * S
                    yv = ys[g % 2]
                    if not tiny_out and g >= 2:
                        vector.wait_ge(out_sem, 16 * (g - 1))
                    for w in range(W):
                        vector.tensor_scalar_mul(
                            v[:, w * S:(w + 1) * S], rb[:], dc[:, t0 + w:t0 + w + 1]
                        )
                    vector._custom_dve(
                        SELINIT, out=x[:, 0:M], in0=v[:, 0:M],
                        s0=float(q[0]), s1=float(inv[1]), imm2=float(inv[0]),
                    )
                    cur = x[:, 0:M]
                    for k in range(1, NB):
                        dst = yv[:, 0:M] if k % 2 == 1 else x[:, 0:M]
                        vector._custom_dve(
                            SELCHAIN, out=dst, in0=v[:, 0:M], in1=cur,
                            s0=float(q[k]), s1=float(inv[k + 1]),
                        )
                        cur = dst
                    assert (NB - 1) % 2 == 1  # final landed in yv
                    vector.engine_nop().then_inc(grp_done, 1)
                    t0 += W

    from concourse.library_overlay import lower_extended_insts
    lower_extended_insts(nc)
    return nc


def _in_maps(x, q, inv):
    d = x.reshape(B, S).astype(np.float32)
    recip = (np.float32(1.0) / d).astype(np.float32)
    maps = []
    for b in range(B):
        maps.append({
            "recipb": np.ascontiguousarray(np.broadcast_to(recip[b], (P, S))),
            "dcol": np.ascontiguousarray(d[b].reshape(NT, P).T),
        })
    return maps


def kernel(x, q, inv):
    x = np.asarray(x, dtype=np.float32)
    q = np.asarray(q, dtype=np.float32)
    inv = np.asarray(inv, dtype=np.float32)
    assert x.shape == (B, 1, 48, 48)

    key = (q.tobytes(), inv.tobytes())
    if key not in _CACHE:
        _CACHE[key] = _build_nc(q, inv)
    nc = _CACHE[key]

    from concourse.bass_utils import run_bass_kernel_spmd
    res = run_bass_kernel_spmd(nc, _in_maps(x, q, inv), list(range(B)))
    out = np.stack([res.results[b]["out"] for b in range(B)], axis=0)
    return out



# revision 5
# speedup vs baseline: 2.6136x; 2.6136x over previous
"""Trainium2 Bass kernel for nn_DepthEstimationNet (vq_codebook).

reference:  d = x.reshape(B, S);  ratio[b,i,j] = d[b,i] * (1/d[b,j])
            out[b,i,j] = inv[searchsorted(q, ratio, side='right')]
shapes:     x [8,1,48,48] -> out [8, 2304, 2304] fp32 (~170 MB)

Strategy (data-parallel over batch, one batch per NeuronCore):
  - host computes recip = fl32(1/d) per batch (bit-identical to the
    reference's fp32 divide) and replicates it across 128 SBUF partitions.
  - per 128-row tile: v = d_col * recip (same fp32 rounding as the
    reference ratio), then 11 passes of a raw-uop custom DVE op (ANT_CH4)
    that performs FOUR exact select-chain steps per pass:
        s = select(v >= q_k, inv[k+1], s)  x4
    The 8 per-pass constants (4 thresholds + 4 values) are preloaded into
    the DVE's per-slice swap flops from a 64-element Src1 preamble, so a
    pass costs ~1 cycle/element like a single chain step of the 41-step
    1-op-per-threshold formulation (~3.8x fewer DVE passes).
  - pass 0 uses threshold -3e38 to inject the inv[0] default, so its s_in
    stream can be garbage (the v buffer itself) - no init memset.
  - row tiles are processed in groups of 4/4/4/3/3; output DMA of group g
    overlaps compute of group g+1.
"""
import numpy as np

from rawop import TBL, register_chain_op, make_table

S = 2304          # 48*48
P = 128           # partitions
NT = S // P       # 18 row tiles per batch
NB = 40           # thresholds
B = 8             # batch == cores
GROUPS = (4, 4, 4, 3, 3)   # tile-group widths, sum = NT
WMAX = max(GROUPS)
NPASS = 11        # chain passes per group

_CACHE = {}


def _tables(q, inv):
    """NPASS per-pass constant blocks [tA,vA,tB,vB,tC,vC,tD,vD]."""
    LO = np.float32(-3.0e38)
    blocks = [[LO, inv[0], q[0], inv[1], q[1], inv[2], q[2], inv[3]]]
    for p in range(1, 10):
        k = 4 * p - 1
        blocks.append([q[k], inv[k + 1], q[k + 1], inv[k + 2],
                       q[k + 2], inv[k + 3], q[k + 3], inv[k + 4]])
    blocks.append([q[39], inv[40]] * 4)
    assert len(blocks) == NPASS
    return np.concatenate([make_table(b) for b in blocks])  # [NPASS*TBL]


def _build_nc(q, inv, repeat=1, tiny_out=False):
    import concourse.bass as bass
    import concourse.mybir as mybir

    CH4 = register_chain_op()
    f32 = mybir.dt.float32

    nc = bass.Bass()
    r_in = nc.declare_dram_parameter("recipb", [P, S], f32, isOutput=False)
    d_in = nc.declare_dram_parameter("dcol", [P, NT], f32, isOutput=False)
    t_in = nc.declare_dram_parameter("tbls", [P, NPASS * TBL], f32, isOutput=False)
    out_shape = [P, 8] if tiny_out else [S, S]
    y_out = nc.declare_dram_parameter("out", out_shape, f32, isOutput=True)

    NG = len(GROUPS)
    MX = WMAX * S
    with (
        nc.sbuf_tensor("rb", [P, S], f32) as rb,
        nc.sbuf_tensor("dc", [P, NT], f32) as dc,
        nc.sbuf_tensor("tb", [P, NPASS * TBL], f32) as tb,
        nc.sbuf_tensor("vb", [P, TBL + MX], f32) as vb,
        nc.sbuf_tensor("sa", [P, TBL + MX], f32) as sa,
        nc.sbuf_tensor("sb", [P, TBL + MX], f32) as sb,
        nc.sbuf_tensor("sc", [P, TBL + MX], f32) as sc,
        nc.Block() as block,
        nc.semaphore("in_sem") as in_sem,
        nc.semaphore("grp_done") as grp_done,
        nc.semaphore("out_sem") as out_sem,
    ):
        @block.sync
        def _(sync):
            sync.dma_start(out=rb[:], in_=r_in[:]).then_inc(in_sem, 16)
            sync.dma_start(out=dc[:], in_=d_in[:]).then_inc(in_sem, 16)
            sync.dma_start(out=tb[:], in_=t_in[:]).then_inc(in_sem, 16)
            sync.dma_start(out=vb[:, 0:TBL], in_=t_in[:, 0:TBL]).then_inc(in_sem, 16)
            if tiny_out:
                sync.wait_ge(grp_done, NG * repeat)
                sync.dma_start(out=y_out[:], in_=sa[:, TBL:TBL + 8]).then_inc(out_sem, 16)
                sync.wait_ge(out_sem, 16)
            else:
                row0 = 0
                for g, W in enumerate(GROUPS):
                    sync.wait_ge(grp_done, g + 1)
                    dst = y_out[row0:row0 + W * P, :].rearrange(
                        "(w p) s -> p w s", p=P
                    )
                    fin = (sa, sb, sc)[g % 3]
                    src = fin[:, TBL:TBL + W * S].rearrange(
                        "p (w s) -> p w s", s=S
                    )
                    sync.dma_start(out=dst, in_=src).then_inc(out_sem, 16)
                    row0 += W * P
                sync.wait_ge(out_sem, 16 * NG)

        @block.vector
        def _(vector):
            vector.wait_ge(in_sem, 64)
            import contextlib
            rep_ctx = (
                vector.Fori(0, repeat) if repeat > 1 else contextlib.nullcontext()
            )
            with rep_ctx:
                t0 = 0
                for g, W in enumerate(GROUPS):
                    M = W * S
                    rot = (sa, sb, sc)
                    fin = rot[g % 3]          # final (and pass-0) buffer
                    scr = rot[(g + 1) % 3]    # ping-pong scratch
                    if not tiny_out and g >= 2:
                        # fin/scr were DMA sources of groups <= g-2
                        vector.wait_ge(out_sem, 16 * (g - 1))
                    for w in range(W):
                        vector.tensor_scalar_mul(
                            vb[:, TBL + w * S:TBL + (w + 1) * S],
                            rb[:], dc[:, t0 + w:t0 + w + 1]
                        )
                    v_ap = vb[:, TBL:TBL + M]
                    # pass 0: in1 = vb (table0 + garbage s, masked by -3e38)
                    vector._custom_dve(
                        CH4, out=fin[:, TBL:TBL + M], in0=v_ap,
                        in1=vb[:, 0:TBL + M],
                    )
                    cur = fin
                    for p in range(1, NPASS):
                        nxt = scr if cur is fin else fin
                        vector.tensor_copy(
                            out=cur[:, 0:TBL],
                            in_=tb[:, p * TBL:(p + 1) * TBL],
                        )
                        vector._custom_dve(
                            CH4, out=nxt[:, TBL:TBL + M], in0=v_ap,
                            in1=cur[:, 0:TBL + M],
                        )
                        cur = nxt
                    assert cur is fin  # NPASS odd: final lands in pass-0's buffer
                    vector.engine_nop().then_inc(grp_done, 1)
                    t0 += W

    from concourse.library_overlay import lower_extended_insts
    lower_extended_insts(nc)
    return nc


def _in_maps(x, q, inv):
    d = x.reshape(B, S).astype(np.float32)
    recip = (np.float32(1.0) / d).astype(np.float32)
    tbl = _tables(np.asarray(q, np.float32), np.asarray(inv, np.float32))
    tblb = np.ascontiguousarray(np.broadcast_to(tbl, (P, tbl.size)))
    maps = []
    for b in range(B):
        maps.append({
            "recipb": np.ascontiguousarray(np.broadcast_to(recip[b], (P, S))),
            "dcol": np.ascontiguousarray(d[b].reshape(NT, P).T),
            "tbls": tblb,
        })
    return maps


def kernel(x, q, inv):
    x = np.asarray(x, dtype=np.float32)
    q = np.asarray(q, dtype=np.float32)
    inv = np.asarray(inv, dtype=np.float32)
    assert x.shape == (B, 1, 48, 48)

    key = (q.tobytes(), inv.tobytes())
    if key not in _CACHE:
        _CACHE[key] = _build_nc(q, inv)
    nc = _CACHE[key]

    from concourse.bass_utils import run_bass_kernel_spmd
    res = run_bass_kernel_spmd(nc, _in_maps(x, q, inv), list(range(B)))
    out = np.stack([res.results[b]["out"] for b in range(B)], axis=0)
    return out
